# revision 15
# baseline (speedup 1.0000x reference)
"""Mixture-of-Experts (top-2 of 8) on 8 Trainium2 NeuronCores.

Load-balanced expert-parallel sharding. The gate (softmax top-2,
renormalized) runs on the host (0.4% of FLOPs). Token-expert slots are
packed into a uniform multi-segment layout so one SPMD program serves all
cores while per-core work drops from max-expert-load (1092 here) to near
the mean (1024):

    every core runs the same fixed sequence of segments (sizes s_1..s_k,
    k = 2 or 3), each segment bound to its own weight-set input; which
    expert a segment holds is per-core data. The host planner splits each
    expert's slot list across 2-4 segments, choosing sizes by closed form
    (k=2) or a pruned DP search (k=3) to minimize C = sum(s_i). For the
    reference routing this gives C = 1030 vs the 1024 floor.

Per segment the FFN is the proven streaming structure, now in bf16
(same 1 PE-cycle/row as fp32r, half the HBM traffic — needed because
two weight sets stream per core — rel err ~2e-3 vs 2e-2 budget):

    mm1: H^T[f, t] = sum_d W1[d, f] * X^T[d, t]   (lhsT = W1 tile, natural)
    act: H^T = gelu_erf(H^T + b1)                 (bias along partitions)
    mm2: Y^T[d, t] = sum_f W2[f, d] * H^T[f, t]   (lhsT = W2 tile, natural)

F(=4096) streams in 8 blocks of 512 per segment; Y^T accumulates across
blocks in SBUF (fp32 DVE adds). The host combines Y with the top-2 gate
weights.
"""

import os
import sys

import numpy as np

if "/opt/trn_rl_repo" not in sys.path:
    sys.path.insert(0, "/opt/trn_rl_repo")

# A JAX_PLATFORMS=cpu pin (used by some reference harnesses) would hide the
# NeuronCores from the PJRT execute path; drop it while jax is still
# unimported so jax.devices() sees the axon trn2 devices.
if "jax" not in sys.modules and os.environ.get("JAX_PLATFORMS") == "cpu":
    del os.environ["JAX_PLATFORMS"]

import ml_dtypes

BF16 = ml_dtypes.bfloat16
P = 128
TOP_K = 2


def _routing(xf, gate_w):
    """Top-2 expert ids and renormalized softmax scores, matching
    jax.nn.softmax + jax.lax.top_k (ties -> lower index) semantics."""
    T = xf.shape[0]
    logits = (xf.astype(np.float64) @ gate_w.astype(np.float64))  # [T, E]
    i1 = np.argmax(logits, axis=-1)
    tok = np.arange(T)
    masked = logits.copy()
    masked[tok, i1] = -np.inf
    i2 = np.argmax(masked, axis=-1)
    l1 = logits[tok, i1]
    l2 = logits[tok, i2]
    # renormalized top-2 softmax: full softmax denominator cancels
    e2 = np.exp(l2 - l1)
    s1 = 1.0 / (1.0 + e2)
    s2 = e2 / (1.0 + e2)
    idx = np.stack([i1, i2], axis=1).astype(np.int32)
    scores = np.stack([s1, s2], axis=1).astype(np.float32)
    return idx, scores


def _chunks(C):
    """Split C into near-equal chunks <=512 (PSUM bank limit), each >=256
    when possible so LDWEIGHTS and drain overheads stay hidden."""
    if C <= 512:
        return [(0, C)]
    n = -(-C // 512)
    # largest chunk first (its compute covers later chunks' DMA), then the
    # remaining chunks ascending; every chunk >=256 when C allows
    sizes = []
    rem = C
    for i in range(n):
        left = n - i - 1
        s = min(512, rem - 256 * left)
        sizes.append(s)
        rem -= s
    sizes = sizes[:1] + sorted(sizes[1:])
    out, c0 = [], 0
    for s in sizes:
        out.append((c0, s))
        c0 += s
    return out


def _plan2(counts):
    """Two-segment plan: uniform sizes (u, v), every expert covered by
    exactly two segments. Returns (sizes, assign) with assign[e] a list of
    (kind_index, n_slots)."""
    counts = np.asarray(counts)
    E = len(counts)
    order = np.argsort(counts, kind="stable")
    L = counts[order].astype(int)
    best = None
    for a in range(0, E // 2 + 1):
        b = E - 2 * a
        if a == 0:
            C = int(L[-1]) + int(L[-1]) % 2
            u = C // 2
            u += u % 2
        else:
            u = max(2, -(-int(L[a - 1]) // 2))
            u += u % 2
            v0 = max(2, -(-int(L[-1]) // 2))
            v0 += v0 % 2
            mid = int(L[-1 - a]) if b > 0 else 0
            C = max(u + v0, mid)
            C += C % 2
        if C < u + 2:
            C = u + 2
        if best is None or C < best[0]:
            best = (C, u, a)
    C, u, a = best
    v = C - u
    assign = [None] * E
    for i, e in enumerate(order):
        Le = int(counts[e])
        if i < a:
            kind = (0, 0)
            first = min(u, Le)
        elif i >= E - a:
            kind = (1, 1)
            first = min(v, Le)
        else:
            kind = (0, 1)
            first = min(u, Le)
        assign[e] = [(kind[0], first), (kind[1], Le - first)]
    return (u, v), assign


def _plan3_feasible(s, loads_desc, E, combos):
    """DP: can loads be covered using E segments of each size in s, each
    expert using 2-4 segments? Returns the assignment (list of combos in
    loads_desc order) or None."""
    states = {(0, 0, 0): None}
    layers = [states]
    for Le in loads_desc:
        nxt = {}
        for st in layers[-1]:
            u1, u2, u3 = st
            for (a, b, c, cap) in combos:
                if cap >= Le and u1 + a <= E and u2 + b <= E and u3 + c <= E:
                    ns = (u1 + a, u2 + b, u3 + c)
                    if ns not in nxt:
                        nxt[ns] = (st, (a, b, c))
        if not nxt:
            return None
        layers.append(nxt)
    goal = (E, E, E)
    if goal not in layers[-1]:
        return None
    # backtrack
    picks = []
    st = goal
    for layer in layers[:0:-1]:
        prev, combo = layer[st]
        picks.append(combo)
        st = prev
    picks.reverse()
    return picks


def _plan3(counts, c_bound):
    """Three-segment plan via pruned DP search over segment sizes (must
    beat c_bound). Returns (sizes, assign) like _plan2, or None."""
    counts = np.asarray(counts).astype(int)
    E = len(counts)
    loads_desc = sorted(counts.tolist(), reverse=True)
    mean3 = int(np.mean(counts)) // 3
    lo1, hi1 = max(2, mean3 - 64), mean3 + 40
    lo1 += lo1 % 2  # even sizes only
    best = None
    best_c = c_bound
    for s1 in range(lo1, hi1, 2):
        if s1 * 3 >= best_c:
            break
        for s2 in range(s1, s1 + 120, 2):
            if s1 + 2 * s2 >= best_c:
                break
            # only search below the current best: check feasibility at the
            # largest useful s3, then binary-search the minimum
            hi = best_c - s1 - s2 - 1
            if hi < s2:
                continue
            if _plan3_feasible(
                (s1, s2, hi), loads_desc, E, _combos3(s1, s2, hi)
            ) is None:
                continue
            lo = s2
            while lo < hi:
                mid = (lo + hi) // 2
                if _plan3_feasible(
                    (s1, s2, mid), loads_desc, E, _combos3(s1, s2, mid)
                ) is not None:
                    hi = mid
                else:
                    lo = mid + 1
            C = s1 + s2 + lo
            if best is None or C < best_c:
                best = (C, s1, s2, lo)
                best_c = C
    if best is None:
        return None
    _, s1, s2, s3 = best
    s3 += s3 % 2  # even sizes only
    picks = _plan3_feasible(
        (s1, s2, s3), loads_desc, E, _combos3(s1, s2, s3)
    )
    sizes = (s1, s2, s3)
    # map picks (desc order) back to expert ids, splitting loads into pieces
    order_desc = np.argsort(-counts, kind="stable")
    assign = [None] * E
    for i, e in enumerate(order_desc):
        a, b, c = picks[i]
        caps = [0] * a + [1] * b + [2] * c
        rem = int(counts[e])
        pieces = []
        for k in caps:
            n = min(rem, sizes[k])
            pieces.append((k, n))
            rem -= n
        assert rem == 0
        assign[e] = pieces
    return sizes, assign


def _combos3(s1, s2, s3):
    out = []
    for a in range(5):
        for b in range(5):
            for c in range(5):
                if 2 <= a + b + c <= 4:
                    out.append((a, b, c, a * s1 + b * s2 + c * s3))
    return out


_PLAN_CACHE = {}


def _plan(counts):
    """Pick the best uniform segment plan (2- or 3-segment)."""
    key = tuple(int(c) for c in counts)
    if key in _PLAN_CACHE:
        return _PLAN_CACHE[key]
    sizes2, assign2 = _plan2(counts)
    out = (sizes2, assign2)
    p3 = _plan3(counts, sum(sizes2))
    if p3 is not None and sum(p3[0]) < sum(sizes2):
        out = p3
    _PLAN_CACHE[key] = out
    return out


_BUILD_CACHE = {}


def _build(C, D, F, reps=1):
    """Build the per-core multi-segment Bass module. C is the tuple of
    uniform segment sizes. reps>1 repeats the computation (for
    timing-by-slope)."""
    sizes = tuple(int(s) for s in C)
    key = (sizes, D, F, reps)
    if key in _BUILD_CACHE:
        return _BUILD_CACHE[key]

    from concourse import bacc
    import concourse.tile as tile
    import concourse.mybir as mybir

    f32 = mybir.dt.float32
    bf16 = mybir.dt.bfloat16

    ND = D // P            # 8 d-tiles
    NF = F // P            # 32 f-tiles
    FB = 4                 # f-tiles per weight block
    NB = NF // FB          # 8 blocks
    FBW = FB * P           # 512 f columns per block
    NS = len(sizes)
    sfx = "abcdefgh"[:NS]
    Ct = sum(sizes)

    nc = bacc.Bacc(None)
    xt = nc.dram_tensor("xt", [P, ND, Ct], bf16, kind="ExternalInput")
    w1n = [nc.dram_tensor(f"w1{s}", [D, F], bf16, kind="ExternalInput")
           for s in sfx]
    w2n = [nc.dram_tensor(f"w2{s}", [F, D], bf16, kind="ExternalInput")
           for s in sfx]
    b1n = [nc.dram_tensor(f"b1{s}", [P, NF], f32, kind="ExternalInput")
           for s in sfx]
    b2n = [nc.dram_tensor(f"b2{s}", [P, ND], f32, kind="ExternalInput")
           for s in sfx]
    yt = nc.dram_tensor("yt", [P, ND, Ct], f32, kind="ExternalOutput")
    # natural-layout weights, viewed with the 128-partition dim innermost
    w1 = [t[:].rearrange("(dp p) f -> p dp f", p=P) for t in w1n]
    w2 = [t[:].rearrange("(fp p) d -> p fp d", p=P) for t in w2n]
    segs = []
    off = 0
    for s in sizes:
        segs.append((off, s))
        off += s

    with tile.TileContext(nc) as tc:
        with (
            tc.tile_pool(name="res", bufs=1) as res,
            tc.tile_pool(name="w1p", bufs=2) as w1p,
            tc.tile_pool(name="w2p", bufs=2) as w2p,
            tc.tile_pool(name="hp", bufs=3) as hp,
            tc.tile_pool(name="php", bufs=4, space="PSUM") as php,
            tc.tile_pool(name="pyp", bufs=3, space="PSUM") as pyp,
        ):
            def load_block(si, fb):
                w1_sb = w1p.tile([P, ND, FBW], bf16, tag="w1blk")
                nc.sync.dma_start(
                    w1_sb[:], w1[si][:, :, fb * FBW : (fb + 1) * FBW]
                )
                w2_sb = w2p.tile([P, FB, D], bf16, tag="w2blk")
                nc.sync.dma_start(w2_sb[:], w2[si][:, fb * FB : (fb + 1) * FB, :])
                return w1_sb, w2_sb

            # pipeline-fill: SP sequencer dispatch costs ~650ns per DMA,
            # so lead with the two small pieces the first matmul needs,
            # then bandwidth-sized pieces in demand order.
            xt_sb = res.tile([P, ND, Ct], bf16)
            c00, cn0 = _chunks(sizes[0])[0]
            w1_sb0 = w1p.tile([P, ND, FBW], bf16, tag="w1blk")
            b1_sb = [res.tile([P, NF], f32, name=f"b1{s}") for s in sfx]
            nc.sync.dma_start(w1_sb0[:, :, 0:P], w1[0][:, :, 0:P])
            nc.sync.dma_start(
                xt_sb[:, 0:1, c00 : c00 + cn0], xt[:, 0:1, c00 : c00 + cn0]
            )
            nc.sync.dma_start(
                xt_sb[:, 1:3, c00 : c00 + cn0], xt[:, 1:3, c00 : c00 + cn0]
            )
            nc.sync.dma_start(
                xt_sb[:, 3:5, c00 : c00 + cn0], xt[:, 3:5, c00 : c00 + cn0]
            )
            nc.sync.dma_start(
                xt_sb[:, 5:8, c00 : c00 + cn0], xt[:, 5:8, c00 : c00 + cn0]
            )
            nc.sync.dma_start(w1_sb0[:, :, P : 2 * P], w1[0][:, :, P : 2 * P])
            nc.sync.dma_start(b1_sb[0][:], b1n[0][:])
            nc.sync.dma_start(
                w1_sb0[:, :, 2 * P : 4 * P], w1[0][:, :, 2 * P : 4 * P]
            )
            w2_sb0 = w2p.tile([P, FB, D], bf16, tag="w2blk")
            nc.sync.dma_start(w2_sb0[:], w2[0][:, 0:FB, :])
            blk0 = (w1_sb0, w2_sb0)
            b2_sb = [res.tile([P, ND], f32, name=f"b2{s}") for s in sfx]
            nc.sync.dma_start(b2_sb[0][:], b2n[0][:])
            for si in range(1, NS):
                nc.sync.dma_start(b1_sb[si][:], b1n[si][:])
                nc.sync.dma_start(b2_sb[si][:], b2n[si][:])
            if cn0 < sizes[0]:
                nc.sync.dma_start(
                    xt_sb[:, :, cn0 : sizes[0]], xt[:, :, cn0 : sizes[0]]
                )
            for (base, size) in segs[1:]:
                for (c0, cn) in _chunks(size):
                    nc.sync.dma_start(
                        xt_sb[:, :, base + c0 : base + c0 + cn],
                        xt[:, :, base + c0 : base + c0 + cn],
                    )
            y_sb = []
            for dp in range(ND):
                y_sb.append(res.tile([P, Ct], f32, name=f"y{dp}"))

            for rep in range(reps):
                _body(nc, tc, segs, D, F, load_block, hp, php, pyp,
                      xt_sb, y_sb, b1_sb, b2_sb, yt,
                      blk0 if rep == 0 else None)

    nc.compile()
    _BUILD_CACHE[key] = nc
    return nc


def _body(nc, tc, segs, D, F, load_block, hp, php, pyp,
          xt_sb, y_sb, b1_sb, b2_sb, yt, blk0=None):
    import concourse.mybir as mybir

    f32 = mybir.dt.float32
    bf16 = mybir.dt.bfloat16
    Gelu = mybir.ActivationFunctionType.Gelu
    ND = D // P
    NF = F // P
    FB = 4
    NB = NF // FB

    for si, (base, size) in enumerate(segs):
        for fb in range(NB):
            if si == 0 and fb == 0 and blk0 is not None:
                w1_sb, w2_sb = blk0
            else:
                w1_sb, w2_sb = load_block(si, fb)

            for (c0, cn) in _chunks(size):
                t0 = base + c0
                h_sb = hp.tile([P, FB, 512], bf16)
                for fi in range(FB):
                    ph = php.tile([P, 512], f32)
                    for dp in range(ND):
                        nc.tensor.matmul(
                            ph[:, :cn],
                            lhsT=w1_sb[:, dp, fi * P : (fi + 1) * P],
                            rhs=xt_sb[:, dp, t0 : t0 + cn],
                            start=(dp == 0),
                            stop=(dp == ND - 1),
                        )
                    ft = fb * FB + fi
                    nc.scalar.activation(
                        h_sb[:, fi, :cn],
                        ph[:, :cn],
                        Gelu,
                        bias=b1_sb[si][:, ft : ft + 1],
                        scale=1.0,
                    )
                for dp in range(ND):
                    py = pyp.tile([P, 512], f32)
                    for fi in range(FB):
                        nc.tensor.matmul(
                            py[:, :cn],
                            lhsT=w2_sb[:, fi, dp * P : (dp + 1) * P],
                            rhs=h_sb[:, fi, :cn],
                            start=(fi == 0),
                            stop=(fi == FB - 1),
                        )
                    if fb == 0:
                        # fold the b2 bias in up front (per-partition scalar)
                        nc.vector.tensor_scalar_add(
                            y_sb[dp][:, t0 : t0 + cn],
                            py[:, :cn],
                            b2_sb[si][:, dp : dp + 1],
                        )
                    else:
                        nc.vector.tensor_add(
                            y_sb[dp][:, t0 : t0 + cn],
                            y_sb[dp][:, t0 : t0 + cn],
                            py[:, :cn],
                        )
                    if fb == NB - 1:
                        nc.sync.dma_start(
                            yt[:, dp, t0 : t0 + cn],
                            y_sb[dp][:, t0 : t0 + cn],
                        )


def _run(nc, in_maps):
    from concourse.bass_utils import run_bass_kernel_spmd

    return run_bass_kernel_spmd(nc, in_maps, core_ids=list(range(len(in_maps))))


def _prepare(x, gate_w, w1, b1, w2, b2, routing=None):
    """Routing + per-core input construction. Returns
    (nc, in_maps, slots, core_segs, C) with C = (u, v)."""
    B, S, D = x.shape
    E, _, F = w1.shape
    T = B * S
    xf = np.ascontiguousarray(x.reshape(T, D), dtype=np.float32)

    idx, scores = routing if routing is not None else _routing(xf, gate_w)

    slots = []
    wts = []
    for e in range(E):
        m1 = idx[:, 0] == e
        m2 = idx[:, 1] == e
        toks = np.concatenate([np.nonzero(m1)[0], np.nonzero(m2)[0]])
        ws = np.concatenate([scores[m1, 0], scores[m2, 1]])
        slots.append(toks)
        wts.append(ws)

    sizes, assign = _plan([len(t) for t in slots])
    C = tuple(sizes)
    NS = len(C)
    sfx = "abcdefgh"[:NS]
    Ct = sum(C)
    bases = np.concatenate([[0], np.cumsum(C)[:-1]]).astype(int)

    # expert pieces per segment kind -> (expert, slot_offset, n)
    pieces = [[] for _ in range(NS)]
    for e in range(E):
        off = 0
        for kind, n in assign[e]:
            pieces[kind].append((e, off, n))
            off += n
    assert all(len(p) == E for p in pieces), [len(p) for p in pieces]

    nc = _build(C, D, F)

    ND, NF = D // P, F // P
    xb = xf.astype(BF16)
    w1b = w1.astype(BF16)
    w2b = w2.astype(BF16)
    in_maps = []
    core_segs = []
    for c in range(E):
        xt = np.zeros((P, ND, Ct), BF16)
        im = {"xt": xt}
        segs_c = []
        for k in range(NS):
            e, o, n = pieces[k][c]
            toks = slots[e][o : o + n]
            if n:
                # [n, D] -> [D, n] -> [ND, P, n] -> [P, ND, n]
                xt[:, :, bases[k] : bases[k] + n] = (
                    xb[toks].T.reshape(ND, P, n).transpose(1, 0, 2)
                )
            s = sfx[k]
            im[f"w1{s}"] = np.ascontiguousarray(w1b[e])
            im[f"w2{s}"] = np.ascontiguousarray(w2b[e])
            im[f"b1{s}"] = np.ascontiguousarray(b1[e].reshape(NF, P).T)
            im[f"b2{s}"] = np.ascontiguousarray(b2[e].reshape(ND, P).T)
            segs_c.append((toks, wts[e][o : o + n], int(bases[k])))
        in_maps.append(im)
        core_segs.append(tuple(segs_c))
    return nc, in_maps, slots, core_segs, C


def _combine(results, core_segs, T, D):
    out = np.zeros((T, D), np.float32)
    for c, segs in enumerate(core_segs):
        y = results[c]["yt"]  # [P, ND, Ct]
        y = y.transpose(1, 0, 2).reshape(D, -1)  # [D, Ct]
        for toks, ws, base in segs:
            n = len(toks)
            if n:
                out[toks] += ws[:, None] * y[:, base : base + n].T
    return out


_MAX_C = 2048  # SBUF limit for the capacity-resident layout


def kernel(x, gate_w, w1, b1, w2, b2):
    x, gate_w, w1, b1, w2, b2 = (
        np.asarray(a) for a in (x, gate_w, w1, b1, w2, b2)
    )
    B, S, D = x.shape
    T = B * S
    xf = np.ascontiguousarray(x.reshape(T, D), dtype=np.float32)
    routing = _routing(xf, gate_w)
    counts = np.bincount(routing[0].ravel(), minlength=w1.shape[0])
    sizes, _ = _plan(counts)
    if sum(sizes) > _MAX_C and S % 2 == 0:
        # pathologically skewed routing: halve the token set and recurse
        h = S // 2
        lo = kernel(x[:, :h], gate_w, w1, b1, w2, b2)
        hi = kernel(x[:, h:], gate_w, w1, b1, w2, b2)
        return np.concatenate([lo, hi], axis=1)
    nc, in_maps, slots, core_segs, C = _prepare(
        x, gate_w, w1, b1, w2, b2, routing
    )
    res = _run(nc, in_maps)
    out = _combine(res.results, core_segs, T, D)
    return out.reshape(B, S, D)


def timed_run(nc, in_maps, iters=20):
    """Time warm executions with device-resident inputs. Returns
    (per_iter_seconds_list, results). Mirrors bass2jax.run_bass_via_pjrt's
    multi-core branch but without donation so buffers can be reused."""
    import time

    import jax
    import numpy as _np
    from jax.sharding import Mesh, NamedSharding, PartitionSpec
    from jax.experimental.shard_map import shard_map
    from concourse import bass2jax, mybir
    from concourse.bass2jax import _bass_exec_p, install_neuronx_cc_hook

    install_neuronx_cc_hook()
    n_cores = len(in_maps)

    partition_name = nc.partition_id_tensor.name if nc.partition_id_tensor else None
    in_names, out_names, out_avals, zero_outs = [], [], [], []
    for alloc in nc.m.functions[0].allocations:
        if not isinstance(alloc, mybir.MemoryLocationSet):
            continue
        name = alloc.memorylocations[0].name
        if alloc.kind == "ExternalInput":
            if name != partition_name:
                in_names.append(name)
        elif alloc.kind == "ExternalOutput":
            shape = tuple(alloc.tensor_shape)
            dtype = mybir.dt.np(alloc.dtype)
            out_names.append(name)
            out_avals.append(jax.core.ShapedArray(shape, dtype))
            zero_outs.append(_np.zeros(shape, dtype))
    n_params = len(in_names)
    all_in_names = in_names + out_names
    if partition_name is not None:
        all_in_names.append(partition_name)

    def _body(*args):
        operands = list(args)
        if partition_name is not None:
            operands.append(bass2jax.partition_id_tensor())
        outs = _bass_exec_p.bind(
            *operands,
            out_avals=tuple(out_avals),
            in_names=tuple(all_in_names),
            out_names=tuple(out_names),
            lowering_input_output_aliases=(),
            sim_require_finite=True,
            sim_require_nnan=True,
            nc=nc,
        )
        return tuple(outs)

    devices = jax.devices()[:n_cores]
    mesh = Mesh(_np.asarray(devices), ("core",))
    n_outs = len(out_names)
    in_specs = (PartitionSpec("core"),) * (n_params + n_outs)
    out_specs = (PartitionSpec("core"),) * n_outs
    sharded = jax.jit(
        shard_map(_body, mesh=mesh, in_specs=in_specs, out_specs=out_specs,
                  check_rep=False),
        keep_unused=True,
    )
    sh = NamedSharding(mesh, PartitionSpec("core"))
    concat_in = [
        jax.device_put(
            _np.concatenate([_np.asarray(in_maps[c][nm]) for c in range(n_cores)],
                            axis=0), sh)
        for nm in in_names
    ]
    concat_zeros = [
        jax.device_put(_np.zeros((n_cores * z.shape[0], *z.shape[1:]), z.dtype), sh)
        for z in zero_outs
    ]
    # warm-up
    out_arrs = sharded(*concat_in, *concat_zeros)
    jax.block_until_ready(out_arrs)
    times = []
    for _ in range(iters):
        t0 = time.perf_counter()
        out_arrs = sharded(*concat_in, *concat_zeros)
        jax.block_until_ready(out_arrs)
        times.append(time.perf_counter() - t0)
    results = [
        {nm: _np.asarray(out_arrs[i]).reshape(n_cores, *out_avals[i].shape)[c]
         for i, nm in enumerate(out_names)}
        for c in range(n_cores)
    ]
    return times, results
